# revision 4
# baseline (speedup 1.0000x reference)
"""Trainium2 Bass kernel for nn_DE3 (histogram_binning + entropy).

Full input: img [16, 2048, 2048] f32 with values in [0, 256).
reference = B * (8 - res), res = -sum p log2 p, p = bincount(floor(img)) / (H*W).

Strategy (8 NeuronCores, data parallel):
  - Split the 64Mi elements into 8 shards of 8Mi (one per core).
  - Per core, per [128, F] tile: derive hi = floor(x/16) (float round trick)
    and mb = x - 16*hi - 8 in [-8, 8), both as fp16.
  - Ladders (0/1 indicators): lhi_m = (hi >= m) for m=0..15 (m=0 threshold
    -1e4 -> all ones), llo_n = (mb >= n-8). Split across DVE (is_ge, 4x
    mode on fp16) and ACT (Sign -> +-1, un-mixed exactly on the host using
    the ones-row margins).
  - PE: for each slab of 8 columns (1024 elements), one [128,128]x[128,128]
    matmul accumulates J contributions into a single PSUM tile; stationary
    column 8*m+g = hi-ladder m of group g, moving column 8*n+g = lo-ladder n
    of group g. Diagonal blocks [8m+g, 8n+g] hold J_g[m,n] =
    #{hi>=m AND lo>=n}; off-diagonal blocks are unused.
  - Host: sum J over cores/groups, unmix ACT's +-1 rows/cols, 2-D finite
    difference -> 256 counts, then the trivial entropy epilogue.
"""

import numpy as np

import concourse.bass as bass
import concourse.mybir as mybir
from concourse.tile import TileContext
from concourse.bass_utils import run_bass_kernel_spmd

P = 128          # SBUF partitions
F = 1024         # free-dim elements per tile
N_CORES = 8
NHI = 16         # hi (coarse) bins
NLO = 16         # lo (fine) bins within a block
GRP = 8          # column-groups packed per matmul (GRP*NHI = 128 = M)
S = F // GRP     # matmul slabs per tile
assert NHI * NLO == 256

# ladder indices produced on the ACT engine as sign() in {-1,+1}
ACT_HI = (12, 13, 14, 15)
ACT_LO = (12, 13, 14, 15)

_BIG = float(3 * 2**22)  # 1.5*2^23: keeps t in [2^23, 2^24) where ulp = 1

_MAX_WAITS = 1  # this walrus build supports at most 1 sync-wait per instruction


def _split_excess_waits(nc):
    """Walrus in this container rejects instructions with >_MAX_WAITS sync-wait
    commands (Tile's tail drain can carry many). Move excess waits onto
    same-engine NoOp instructions inserted just before the offender."""
    n_split = 0
    for f in nc.m.functions:
        for bb in f.blocks:
            out = []
            for ins in bb.instructions:
                si = getattr(ins, "sync_info", None)
                waits = list(si.on_wait) if si is not None and si.on_wait else []
                if len(waits) > _MAX_WAITS:
                    extra, keep = waits[:-_MAX_WAITS], waits[-_MAX_WAITS:]
                    for ci in range(0, len(extra), _MAX_WAITS):
                        chunk = extra[ci : ci + _MAX_WAITS]
                        nop = mybir.InstNoOp(
                            name=f"{ins.name}-wsplit{ci}",
                            engine=ins.engine,
                            sync_info=mybir.SyncInfo(on_wait=chunk, on_update=[]),
                        )
                        out.append(nop)
                        n_split += 1
                    si.on_wait = keep
                out.append(ins)
            bb.instructions = out
    return n_split


def _register_const(nc, value, dtype=mybir.dt.float32):
    if (dtype, value) in nc.const_aps.aps:
        return
    t = nc.alloc_sbuf_tensor(f"const-{dtype.name}-{value}", [128, 1], dtype)
    nc.gpsimd.memset(t.ap(), value)
    nc.const_aps.aps[(dtype, value)] = t.ap()


def build_nc(n_tiles: int, layout: str = "smajor"):
    """Input [n_tiles*P, S, GRP] f32 -> output J accumulator [128, 128] f32."""
    nc = bass.Bass()
    # const APs for every ACT-engine bias used below
    _register_const(nc, -_BIG)
    for m in ACT_HI:
        _register_const(nc, -(float(m) - 0.5))
    for n in ACT_LO:
        _register_const(nc, -float(n - 8))
    nc.all_engine_barrier()
    x_in = nc.declare_dram_parameter(
        "x", [n_tiles * P, S, GRP], mybir.dt.float32, isOutput=False
    )
    j_out = nc.declare_dram_parameter("j", [P, P], mybir.dt.float32, isOutput=True)

    dt = mybir.dt
    op = mybir.AluOpType

    with TileContext(nc) as tc:
        with (
            tc.tile_pool(name="data", bufs=3) as dpool,
            tc.tile_pool(name="small", bufs=2) as spool,
            tc.tile_pool(name="lad", bufs=2) as lpool,
            tc.tile_pool(name="psum", bufs=1, space="PSUM") as ppool,
            tc.tile_pool(name="outp", bufs=1) as opool,
        ):
            jt = ppool.tile([P, P], dt.float32)
            for it in range(n_tiles):
                x = dpool.tile([P, S, GRP], dt.float32, tag="x")
                nc.sync.dma_start(out=x[:], in_=x_in[it * P : (it + 1) * P, :, :])
                # xb = x - 8 (f32); carries the -0.5 through /16 for the
                # floor-by-round trick (BIG-0.5 is not representable).
                xb = dpool.tile([P, S, GRP], dt.float32, tag="xb")
                nc.vector.tensor_scalar(
                    out=xb[:], in0=x[:], scalar1=-8.0, scalar2=None, op0=op.add
                )
                # t = xb/16 + BIG -> RN: BIG + floor(x/16)  (ties at x=16q)
                t = dpool.tile([P, S, GRP], dt.float32, tag="t")
                nc.vector.tensor_scalar(
                    out=t[:], in0=xb[:], scalar1=1.0 / 16.0, scalar2=_BIG,
                    op0=op.mult, op1=op.add,
                )
                # hi = t - BIG in [0,16), exact small int -> fp16 (ACT engine)
                hi8 = spool.tile([P, S, GRP], dt.float16, tag="hi8")
                nc.scalar.add(hi8[:], t[:], -_BIG)
                # mb = xb - 16*hi = x - 16*hi - 8 in [-8, 8) -> fp16
                mb8 = spool.tile([P, S, GRP], dt.float16, tag="mb8")
                nc.vector.scalar_tensor_tensor(
                    out=mb8[:], in0=hi8[:], scalar=-16.0, in1=xb[:],
                    op0=op.mult, op1=op.add,
                )

                # ladder tiles; layout decides how the matmul slices them
                if layout == "smajor":
                    lhi = lpool.tile([P, S, NHI * GRP], dt.float16, tag="lhi")
                    llo = lpool.tile([P, S, NLO * GRP], dt.float16, tag="llo")
                    hi_slice = lambda m: lhi[:, :, GRP * m : GRP * (m + 1)]
                    lo_slice = lambda n: llo[:, :, GRP * n : GRP * (n + 1)]
                    mm_ops = lambda s: (lhi[:, s, :], llo[:, s, :])
                else:  # "imajor"
                    lhi = lpool.tile([P, NHI, S, GRP], dt.float16, tag="lhi")
                    llo = lpool.tile([P, NLO, S, GRP], dt.float16, tag="llo")
                    hi_slice = lambda m: lhi[:, m, :, :]
                    lo_slice = lambda n: llo[:, n, :, :]
                    mm_ops = lambda s: (lhi[:, :, s, :], llo[:, :, s, :])

                for m in range(NHI):
                    if m in ACT_HI:
                        # sign(hi - (m - 0.5)) in {-1,+1}: hi is integer-valued
                        nc.scalar.sign(hi_slice(m), hi8[:], bias=-(float(m) - 0.5))
                    else:
                        thr = -1.0e4 if m == 0 else float(m)
                        nc.vector.tensor_scalar(
                            out=hi_slice(m), in0=hi8[:], scalar1=thr,
                            scalar2=None, op0=op.is_ge,
                        )
                for n in range(NLO):
                    if n in ACT_LO:
                        # sign(mb - (n-8)): 0 only on exact boundary (rare)
                        nc.scalar.sign(lo_slice(n), mb8[:], bias=-float(n - 8))
                    else:
                        thr = -1.0e4 if n == 0 else float(n - 8)
                        nc.vector.tensor_scalar(
                            out=lo_slice(n), in0=mb8[:], scalar1=thr,
                            scalar2=None, op0=op.is_ge,
                        )

                for s in range(S):
                    lw, mv = mm_ops(s)
                    nc.tensor.matmul(
                        jt[:, :],
                        lhsT=lw,
                        rhs=mv,
                        start=(it == 0 and s == 0),
                        stop=(it == n_tiles - 1 and s == S - 1),
                    )

            jsb = opool.tile([P, P], dt.float32)
            nc.vector.tensor_copy(out=jsb[:], in_=jt[:, :])
            nc.sync.dma_start(out=j_out[:, :], in_=jsb[:])
    _split_excess_waits(nc)
    return nc


def _counts_from_j(j128: np.ndarray) -> np.ndarray:
    """Accumulated [128,128] PSUM image -> 256 bin counts (bin = 16*hi + lo)."""
    # diagonal blocks: J_meas[m, n] = sum_g j128[GRP*m + g, GRP*n + g]
    jm = np.zeros((NHI, NLO), dtype=np.float64)
    for g in range(GRP):
        jm += j128[g::GRP, g::GRP]
    # unmix ACT's +-1 rows/cols: a = 2*l - 1  =>  l = (a + 1) / 2.
    # row/col 0 are plain ones, so margins are available exactly.
    alpha_r = np.array([0.5 if m in ACT_HI else 1.0 for m in range(NHI)])
    beta_r = np.array([0.5 if m in ACT_HI else 0.0 for m in range(NHI)])
    alpha_c = np.array([0.5 if n in ACT_LO else 1.0 for n in range(NLO)])
    beta_c = np.array([0.5 if n in ACT_LO else 0.0 for n in range(NLO)])
    sa = jm[:, 0].copy()   # sum_e a_m(e)   (col 0 is plain ones)
    sb = jm[0, :].copy()   # sum_e b_n(e)
    ne = jm[0, 0]          # element count
    J = (
        np.outer(alpha_r, alpha_c) * jm
        + np.outer(alpha_r * sa, beta_c)
        + np.outer(beta_r, alpha_c * sb)
        + np.outer(beta_r, beta_c) * ne
    )
    # 2-D finite difference of the cumulative-count matrix
    Jp = np.zeros((NHI + 1, NLO + 1), dtype=np.float64)
    Jp[:NHI, :NLO] = J
    A = Jp[:NHI, :] - Jp[1:, :]
    c2 = A[:, :NLO] - A[:, 1:]
    return c2.reshape(256)


def kernel(img: np.ndarray) -> np.ndarray:
    img = np.asarray(img, dtype=np.float32)
    B, H, W = img.shape
    flat = img.reshape(-1)
    n = flat.size
    assert n % (N_CORES * P * F) == 0
    shard = n // N_CORES
    n_tiles = shard // (P * F)

    nc = build_nc(n_tiles)
    in_maps = [
        {"x": flat[i * shard : (i + 1) * shard].reshape(n_tiles * P, S, GRP)}
        for i in range(N_CORES)
    ]
    res = run_bass_kernel_spmd(nc, in_maps, list(range(N_CORES)))
    j128 = np.zeros((P, P), dtype=np.float64)
    for r in res.results:
        j128 += np.asarray(r["j"], dtype=np.float64)

    counts = _counts_from_j(j128)
    temp = float(H * W)
    p = counts / temp
    with np.errstate(divide="ignore", invalid="ignore"):
        terms = np.where(p > 0, p * np.log2(np.where(p > 0, p, 1.0)), 0.0)
    ent = -terms.sum()
    out = np.float32(B * (8.0 - ent))
    return np.asarray(out, dtype=np.float32)
